# revision 2
# baseline (speedup 1.0000x reference)
"""LoRA-XS Linear fused kernel for 8 TRN2 NeuronCores.

out[b,s,o] = x @ (W + U @ sigma @ R @ Vt)^T + bias

Strategy:
  - Host: fold the rank-64 LoRA delta into W (tiny: ~0.5 GFLOP), round
    x / W_eff to fp32r (e8m11, bit-exact with the PE's own rounding),
    and lay out operands k-major for the tensor engine.
  - Device: 8-way data-parallel over the 8192 rows; each core computes
    a 1024x2048 @ 2048x2048 matmul with fp32r 1-pass matmuls (1 cyc/row
    at FD=512, 4x the native fp32 rate) accumulating in fp32 PSUM, plus
    a fused bias add on PSUM eviction.

Shapes (hardcoded): x (4, 2048, 2048) f32, weight (2048, 2048) f32,
bias (2048,) f32, U (2048, 64), sigma/R (64, 64), Vt (64, 2048).
"""

import sys

sys.path.insert(0, "/opt/trn_rl_repo")

import numpy as np

import concourse.bass as bass
import concourse.bacc as bacc
import concourse.mybir as mybir
import concourse.tile as tile
from concourse.bass_utils import run_bass_kernel_spmd

F32 = mybir.dt.float32
F32R = mybir.dt.float32r

ALPHA = 1.0
NCORES = 8
P = 128
B, S, D_IN, D_OUT = 4, 2048, 2048, 2048
ROWS = B * S  # 8192
ROWS_PER_CORE = ROWS // NCORES  # 1024
MT = ROWS_PER_CORE // P  # 8 m-tiles per core
KT = D_IN // P  # 16 k-tiles
NFD = 512  # matmul free dim (one PSUM bank of fp32)
NT = D_OUT // NFD  # 4 n-tiles

_CACHE = {}


def _round_fp32r(a: np.ndarray) -> np.ndarray:
    """RNE-round fp32 to the PE's fp32r (e8m11) — matches walrus
    fp32_to_fp32r bit-for-bit (probe-verified on hardware)."""
    u = np.ascontiguousarray(a).view(np.uint32)
    r = (u + np.uint32(0x7FF) + ((u >> np.uint32(12)) & np.uint32(1))) & np.uint32(
        0xFFFFF000
    )
    return r.view(np.float32)


def _build():
    nc = bacc.Bacc(None, target_bir_lowering=False, debug=False)
    xt = nc.dram_tensor("xt", [P, MT, KT, P], F32R, kind="ExternalInput").ap()
    wt = nc.dram_tensor("wt", [P, KT, D_OUT], F32R, kind="ExternalInput").ap()
    bias = nc.dram_tensor("bias", [D_OUT], F32, kind="ExternalInput").ap()
    out = nc.dram_tensor("out", [P, MT, D_OUT], F32, kind="ExternalOutput").ap()

    with tile.TileContext(nc) as tc:
        with (
            tc.tile_pool(name="const", bufs=1) as const,
            tc.tile_pool(name="wpool", bufs=KT) as wpool,
            tc.tile_pool(name="xpool", bufs=3) as xpool,
            tc.tile_pool(name="opool", bufs=3) as opool,
            tc.tile_pool(name="psum", bufs=2 * NT, space="PSUM") as psum,
        ):
            # bias broadcast across all 128 partitions (step-0 partition AP)
            bias_bc = const.tile([P, D_OUT], F32)
            bias_ap = bass.AP(
                tensor=bias.tensor,
                offset=bias.offset,
                ap=[[0, P], [1, D_OUT]],
            )
            nc.sync.dma_start(out=bias_bc[:], in_=bias_ap)

            # W resident in SBUF, one tile per k-slice for fine-grained deps
            w_tiles = []
            for kk in range(KT):
                w_kk = wpool.tile([P, D_OUT], F32R, name=f"w_{kk}", tag="w")
                nc.sync.dma_start(out=w_kk[:], in_=wt[:, kk, :])
                w_tiles.append(w_kk)

            for mm in range(MT):
                x_mm = xpool.tile([P, KT, P], F32R, name=f"x_{mm}", tag="x")
                nc.sync.dma_start(out=x_mm[:], in_=xt[:, mm, :, :])

                accs = [
                    psum.tile([P, NFD], F32, name=f"acc_{mm}_{nn}", tag="acc")
                    for nn in range(NT)
                ]
                for kk in range(KT):
                    for nn in range(NT):
                        nc.tensor.matmul(
                            accs[nn][:],
                            x_mm[:, kk, :],
                            w_tiles[kk][:, nn * NFD : (nn + 1) * NFD],
                            start=(kk == 0),
                            stop=(kk == KT - 1),
                        )

                o_mm = opool.tile([P, D_OUT], F32, name=f"o_{mm}", tag="o")
                for nn in range(NT):
                    nc.vector.tensor_add(
                        o_mm[:, nn * NFD : (nn + 1) * NFD],
                        accs[nn][:],
                        bias_bc[:, nn * NFD : (nn + 1) * NFD],
                    )
                nc.sync.dma_start(out=out[:, mm, :], in_=o_mm[:])

    nc.compile()
    return nc


def _prepare(x, weight, bias, U, sigma, R, Vt):
    """Host prep: fold LoRA delta, fp32r-round, k-major layouts per core."""
    x = np.asarray(x, dtype=np.float32)
    weight = np.asarray(weight, dtype=np.float32)
    bias = np.asarray(bias, dtype=np.float32)
    U = np.asarray(U, dtype=np.float32)
    sigma = np.asarray(sigma, dtype=np.float32)
    R = np.asarray(R, dtype=np.float32)
    Vt = np.asarray(Vt, dtype=np.float32)

    # Fold LoRA delta into the weight (rank-64: negligible host cost)
    w_eff = weight + ALPHA * ((U @ (sigma @ R)) @ Vt)

    # wt[p, kk, n] = w_eff[n, kk*P + p]
    wt = np.ascontiguousarray(
        _round_fp32r(w_eff).T.reshape(KT, P, D_OUT).transpose(1, 0, 2)
    )
    # xt_c[p, mm, kk, j] = x_core[mm*P + j, kk*P + p]
    xr = _round_fp32r(x.reshape(ROWS, D_IN))
    in_maps = []
    for c in range(NCORES):
        shard = xr[c * ROWS_PER_CORE : (c + 1) * ROWS_PER_CORE]
        xt_c = np.ascontiguousarray(
            shard.reshape(MT, P, KT, P).transpose(3, 0, 2, 1)
        )
        in_maps.append({"xt": xt_c, "wt": wt, "bias": bias})
    return in_maps


def _get_nc():
    if "nc" not in _CACHE:
        _CACHE["nc"] = _build()
    return _CACHE["nc"]


def _gather(core_outs):
    # out_full[c*1024 + mm*128 + p, n] = core_outs[c][p, mm, n]
    stacked = np.stack(core_outs)
    full = stacked.transpose(0, 2, 1, 3).reshape(ROWS, D_OUT)
    return full.reshape(B, S, D_OUT)


def kernel(x, weight, bias, U, sigma, R, Vt):
    in_maps = _prepare(x, weight, bias, U, sigma, R, Vt)
    nc = _get_nc()
    res = run_bass_kernel_spmd(nc, in_maps, list(range(NCORES)))
    return _gather([res.results[c]["out"] for c in range(NCORES)])


# revision 17
# speedup vs baseline: 1.4696x; 1.4696x over previous
"""LoRA-XS Linear fused kernel for 8 TRN2 NeuronCores.

out[b,s,o] = x @ (W + U @ sigma @ R @ Vt)^T + bias

Strategy:
  - Host: fold the rank-64 LoRA delta into W (tiny: ~0.5 GFLOP), round
    x / W_eff to fp32r (e8m11, bit-exact with the PE's own rounding),
    and lay out operands k-major for the tensor engine.
  - Device: 8-way data-parallel over the 8192 rows; each core computes
    a 1024x2048 @ 2048x2048 matmul with fp32r 1-pass matmuls (1 cyc/row
    at FD=512, 4x the native fp32 rate) accumulating in fp32 PSUM, plus
    a fused bias add on PSUM eviction.

Shapes (hardcoded): x (4, 2048, 2048) f32, weight (2048, 2048) f32,
bias (2048,) f32, U (2048, 64), sigma/R (64, 64), Vt (64, 2048).
"""

import sys

sys.path.insert(0, "/opt/trn_rl_repo")

import numpy as np

import concourse.bass as bass
import concourse.bacc as bacc
import concourse.mybir as mybir
import concourse.tile as tile
from concourse.bass_utils import run_bass_kernel_spmd

F32 = mybir.dt.float32
F32R = mybir.dt.float32r

ALPHA = 1.0
NCORES = 8
P = 128
B, S, D_IN, D_OUT = 4, 2048, 2048, 2048
ROWS = B * S  # 8192
ROWS_PER_CORE = ROWS // NCORES  # 1024
MT = ROWS_PER_CORE // P  # 8 m-tiles per core
KT = D_IN // P  # 16 k-tiles
NFD = 512  # matmul free dim (one PSUM bank of fp32)
NT = D_OUT // NFD  # 4 n-tiles

_CACHE = {}


def _round_fp32r(a: np.ndarray) -> np.ndarray:
    """RNE-round fp32 to the PE's fp32r (e8m11) — matches walrus
    fp32_to_fp32r bit-for-bit (probe-verified on hardware)."""
    u = np.ascontiguousarray(a).view(np.uint32)
    r = (u + np.uint32(0x7FF) + ((u >> np.uint32(12)) & np.uint32(1))) & np.uint32(
        0xFFFFF000
    )
    return r.view(np.float32)


def _build():
    nc = bacc.Bacc(None, target_bir_lowering=False, debug=False)
    xt = nc.dram_tensor("xt", [P, MT, KT, P], F32R, kind="ExternalInput").ap()
    wt = nc.dram_tensor("wt", [P, KT, D_OUT], F32R, kind="ExternalInput").ap()
    bias = nc.dram_tensor("bias", [D_OUT], F32, kind="ExternalInput").ap()
    out = nc.dram_tensor("out", [P, MT, D_OUT], F32, kind="ExternalOutput").ap()

    XCH = 4  # k-tiles per x DMA chunk

    with tile.TileContext(nc) as tc:
        with (
            tc.tile_pool(name="const", bufs=1) as const,
            tc.tile_pool(name="xpool", bufs=MT) as xpool,
            tc.tile_pool(name="wpool", bufs=16) as wpool,
            tc.tile_pool(name="opool", bufs=6) as opool,
            tc.tile_pool(name="psum", bufs=MT, space="PSUM") as psum,
        ):
            # x fully resident (64KB/partition); W streamed exactly once
            # through a small ring; out streamed. Four d_out quarter
            # phases, each covering all 8 m-tiles (8 PSUM banks).
            x_tiles = [
                xpool.tile([P, KT, P], F32R, name=f"x_{mm}", tag="x")
                for mm in range(MT)
            ]
            w_tiles = {}

            def load_w(q, kc):
                # one DMA covers two adjacent k-slices of this d_out quarter
                t = wpool.tile([P, 2, NFD], F32R, name=f"w_{q}_{kc}", tag="w")
                nc.sync.dma_start(
                    out=t[:], in_=wt[:, kc : kc + 2, q * NFD : (q + 1) * NFD]
                )
                w_tiles[(q, kc)] = t
                w_tiles[(q, kc + 1)] = t[:, 1, :]
                w_tiles[(q, kc)] = t[:, 0, :]

            # DMA emission in consumption order: x0's chunk + the W slices
            # first (the very first matmul needs only those ~320KB), then
            # the rest of the x ingest interleaved with phase-0's W.
            bias_bc = const.tile([P, D_OUT], F32)
            for kc in range(0, KT, XCH):
                nc.sync.dma_start(
                    out=x_tiles[0][:, kc : kc + XCH, :],
                    in_=xt[:, 0, kc : kc + XCH, :],
                )
                for kk in range(kc, kc + XCH, 2):
                    load_w(0, kk)
                if kc == 0:
                    # bias broadcast across all 128 partitions (step-0
                    # partition AP); needed by the phase-0 evictions,
                    # which gate phase-1's PSUM slots — keep it early
                    bias_ap = bass.AP(
                        tensor=bias.tensor,
                        offset=bias.offset,
                        ap=[[0, P], [1, D_OUT]],
                    )
                    nc.sync.dma_start(out=bias_bc[:], in_=bias_ap)
                for mm in range(1, MT):
                    nc.sync.dma_start(
                        out=x_tiles[mm][:, kc : kc + XCH, :],
                        in_=xt[:, mm, kc : kc + XCH, :],
                    )
            for q in range(1, NT):
                for kk in range(0, KT, 2):
                    load_w(q, kk)

            def evict(accs, q, mm):
                o_q = opool.tile([P, NFD], F32, name=f"o_{q}_{mm}", tag="o")
                nc.vector.tensor_add(
                    o_q[:],
                    accs[mm][:],
                    bias_bc[:, q * NFD : (q + 1) * NFD],
                )
                nc.sync.dma_start(
                    out=out[:, mm, q * NFD : (q + 1) * NFD], in_=o_q[:]
                )

            for q in range(NT):
                accs = [
                    psum.tile([P, NFD], F32, name=f"acc_{q}_{mm}", tag="acc")
                    for mm in range(MT)
                ]
                if q < NT - 1:
                    # k-inner, m-tiles interleaved (follows the W stream)
                    for kk in range(KT):
                        w_kk = w_tiles[(q, kk)]
                        for mm in range(MT):
                            nc.tensor.matmul(
                                accs[mm][:],
                                x_tiles[mm][:, kk, :],
                                w_kk,
                                start=(kk == 0),
                                stop=(kk == KT - 1),
                            )
                    for mm in range(MT):
                        evict(accs, q, mm)
                else:
                    # last phase: finish m-tiles one at a time so the
                    # eviction + store pipeline drains under the matmuls
                    for mm in range(MT):
                        for kk in range(KT):
                            nc.tensor.matmul(
                                accs[mm][:],
                                x_tiles[mm][:, kk, :],
                                w_tiles[(q, kk)],
                                start=(kk == 0),
                                stop=(kk == KT - 1),
                            )
                        evict(accs, q, mm)

    nc.compile()
    return nc


def _prepare(x, weight, bias, U, sigma, R, Vt):
    """Host prep: fold LoRA delta, fp32r-round, k-major layouts per core."""
    x = np.asarray(x, dtype=np.float32)
    weight = np.asarray(weight, dtype=np.float32)
    bias = np.asarray(bias, dtype=np.float32)
    U = np.asarray(U, dtype=np.float32)
    sigma = np.asarray(sigma, dtype=np.float32)
    R = np.asarray(R, dtype=np.float32)
    Vt = np.asarray(Vt, dtype=np.float32)

    # Fold LoRA delta into the weight (rank-64: negligible host cost)
    w_eff = weight + ALPHA * ((U @ (sigma @ R)) @ Vt)

    # wt[p, kk, n] = w_eff[n, kk*P + p]
    wt = np.ascontiguousarray(
        _round_fp32r(w_eff).T.reshape(KT, P, D_OUT).transpose(1, 0, 2)
    )
    # xt_c[p, mm, kk, j] = x_core[mm*P + j, kk*P + p]
    xr = _round_fp32r(x.reshape(ROWS, D_IN))
    in_maps = []
    for c in range(NCORES):
        shard = xr[c * ROWS_PER_CORE : (c + 1) * ROWS_PER_CORE]
        xt_c = np.ascontiguousarray(
            shard.reshape(MT, P, KT, P).transpose(3, 0, 2, 1)
        )
        in_maps.append({"xt": xt_c, "wt": wt, "bias": bias})
    return in_maps


def _get_nc():
    if "nc" not in _CACHE:
        _CACHE["nc"] = _build()
    return _CACHE["nc"]


def _gather(core_outs):
    # out_full[c*1024 + mm*128 + p, n] = core_outs[c][p, mm, n]
    stacked = np.stack(core_outs)
    full = stacked.transpose(0, 2, 1, 3).reshape(ROWS, D_OUT)
    return full.reshape(B, S, D_OUT)


def kernel(x, weight, bias, U, sigma, R, Vt):
    in_maps = _prepare(x, weight, bias, U, sigma, R, Vt)
    nc = _get_nc()
    res = run_bass_kernel_spmd(nc, in_maps, list(range(NCORES)))
    return _gather([res.results[c]["out"] for c in range(NCORES)])
